# revision 1
# baseline (speedup 1.0000x reference)
"""Trainium2 Bass kernel for nn_Attention_layer_12249246728743.

Structure of the reference computation (after untangling the C-order
reshape): per channel c of 512, the 3136 raster positions split into 49
segments of 64 consecutive positions; each segment attends over a 7x7
shifted window of its OWN channel plane (depthwise local attention):

  scores[c,s,p=(i,j)] = sum_d q[c,64s+d] * k[c, win(64s+d, i, j)]
                        + (sum_d q[c,64s+d]) * bias49[p]
  w = softmax_p(scores);  out[c,64s+d] = sum_p w[c,s,p] * v[c, win(...)]

with q/k/v = 1x1 convs of x (k, v on the zero-padded 62x62 domain).

Sharding: channel-parallel across 8 cores. Core r owns channels
{64h + 8r + t : h in 0..7, t in 0..7} (64 channels), so every attention
segment is core-local: no halo, no collectives. x is replicated; weight
rows are gathered per core on host.

On-device per core: 1x1 convs on the PE array (contraction over 512 input
channels, 4 K-tiles), bias folded into PSUM->SBUF eviction on the scalar
engine. Attention runs on DVE+GPSIMD+ACT with a 128-partition layout
(channel, image-half): bf16 window products in DVE 2x mode (odd-shifted
K/V copies keep 4B alignment), fp32 segment-reduces, the rank-1
qsum*bias49 term added once (qsum from an fp32 q path - bf16 there costs
1e-1 scores error), max-subtracted exp on ACT, and two independent bf16
output-accumulator chains (DVE 29 shifts, GPSIMD 20) combined once.
"""

import numpy as np

import concourse.bass as bass
import concourse.mybir as mybir
import concourse.tile as tile
from concourse.bass_utils import run_bass_kernel_spmd

F32 = mybir.dt.float32
BF16 = mybir.dt.bfloat16
AX = mybir.AxisListType
OP = mybir.AluOpType
AF = mybir.ActivationFunctionType

N_CORES = 8
C = 512
H = W = 56
HP = WP = 62          # padded spatial
NPOS = H * W          # 3136
NPAD = HP * WP        # 3844
K = 7
NSH = K * K           # 49 shifts
SEG = 64              # positions per attention segment
NSEG = NPOS // SEG    # 49 segments per channel
CH = 64               # channels per core

# image-half split: half0 = out rows 0..31 (28 segs), half1 = rows 32..55 (21 segs)
H0_ROWS, H1_ROWS = 32, 24
H0_POS, H1_POS = H0_ROWS * W, H1_ROWS * W      # 1792, 1344
H0_SEG, H1_SEG = H0_POS // SEG, H1_POS // SEG  # 28, 21
# padded-row ranges needed per half for the 7-row windows
H0_KROWS, H1_KROWS = H0_ROWS + K - 1, H1_ROWS + K - 1   # 38, 30
KW0, KW1 = H0_KROWS * WP, H1_KROWS * WP                 # 2356, 1860
H1_KOFF = 32 * WP                                       # padded row 32 start


def _build_nc():
    nc = bass.Bass()

    xp = nc.declare_dram_parameter("xp", [C, NPAD], F32, isOutput=False)
    wT = nc.declare_dram_parameter("wT", [C, 3 * CH], F32, isOutput=False)
    bqk = nc.declare_dram_parameter("bqk", [2 * CH, 1], F32, isOutput=False)
    bv = nc.declare_dram_parameter("bv", [CH, 1], F32, isOutput=False)
    b49 = nc.declare_dram_parameter("b49", [128, NSH], F32, isOutput=False)
    out_d = nc.declare_dram_parameter("out", [CH, NPOS], F32, isOutput=True)

    NCHUNK = 512
    chunks = [(c0, min(NCHUNK, NPAD - c0)) for c0 in range(0, NPAD, NCHUNK)]

    with tile.TileContext(nc) as tc:
        with (
            tc.tile_pool(name="persist", bufs=1) as pp,
            tc.tile_pool(name="work", bufs=2) as wp,
            tc.tile_pool(name="psum", bufs=2, space="PSUM") as psp,
        ):
            # ---- loads (batched into few DMAs to bound per-inst sem waits) ----
            xt_all = pp.tile([128, 4 * NPAD], F32, tag="xall", name="xall")
            wt_all = pp.tile([128, 4 * 3 * CH], F32, tag="wall", name="wall")
            for s0 in range(0, NPAD, 1024):
                sn = min(1024, NPAD - s0)
                nc.sync.dma_start(
                    xt_all[:].rearrange("p (k n) -> p k n", k=4)[:, :, s0:s0 + sn],
                    xp[:].rearrange("(k p) n -> p k n", p=128)[:, :, s0:s0 + sn])
            nc.sync.dma_start(
                wt_all[:].rearrange("p (k n) -> p k n", k=4),
                wT[:].rearrange("(k p) n -> p k n", p=128))
            xt = [xt_all[:].rearrange("p (k n) -> p k n", k=4)[:, kt, :]
                  for kt in range(4)]
            wt = [wt_all[:].rearrange("p (k n) -> p k n", k=4)[:, kt, :]
                  for kt in range(4)]
            bqk_s = pp.tile([128, 1], F32, tag="bqk", name="bqk")
            bv_s = pp.tile([CH, 1], F32, tag="bv", name="bv")
            b49_s = pp.tile([128, NSH], F32, tag="b49", name="b49")
            nc.sync.dma_start(bqk_s[:], bqk[:])
            nc.sync.dma_start(bv_s[:], bv[:])
            nc.sync.dma_start(b49_s[:], b49[:])

            # ---- conv staging (channel-major, padded domain) ----
            qs = pp.tile([CH, NPAD], BF16, tag="qs", name="qs")
            qs32 = pp.tile([CH, NPAD], F32, tag="qs32", name="qs32")
            ks = pp.tile([CH, NPAD], BF16, tag="ks", name="ks")
            vs = pp.tile([CH, NPAD], BF16, tag="vs", name="vs")

            # PE pre-touch of xall: keeps every real Matmult at <=1 sem wait
            # (walrus S3_LW codegen rejects multi-wait matmuls).
            dmy = psp.tile([1, 1], F32, tag="dmy", name="dmy")
            nc.tensor.matmul(dmy[:], lhsT=xt_all[0:1, 0:1],
                             rhs=xt_all[0:1, 0:1], start=True, stop=True)

            for c0, n in chunks:
                ps_qk = psp.tile([128, NCHUNK], F32, tag="psqk", name="psqk")
                ps_v = psp.tile([CH, NCHUNK], F32, tag="psv", name="psv")
                for kt in range(4):
                    nc.tensor.matmul(
                        ps_qk[:, :n], lhsT=wt[kt][:, 0:128],
                        rhs=xt[kt][:, c0:c0 + n],
                        start=(kt == 0), stop=(kt == 3))
                    nc.tensor.matmul(
                        ps_v[:, :n], lhsT=wt[kt][:, 128:192],
                        rhs=xt[kt][:, c0:c0 + n],
                        start=(kt == 0), stop=(kt == 3))
                sl = slice(c0, c0 + n)
                nc.scalar.activation(qs[0:CH, sl], ps_qk[0:CH, :n], AF.Identity,
                                     bias=bqk_s[0:CH, :])
                nc.scalar.activation(qs32[0:CH, sl], ps_qk[0:CH, :n],
                                     AF.Identity, bias=bqk_s[0:CH, :])
                nc.scalar.activation(ks[0:CH, sl], ps_qk[CH:128, :n], AF.Identity,
                                     bias=bqk_s[CH:128, :])
                nc.scalar.activation(vs[0:CH, sl], ps_v[0:CH, :n], AF.Identity,
                                     bias=bv_s[:])

            # ---- remap to 128-partition attention layout (bf16) ----
            qa = pp.tile([128, H0_POS], BF16, tag="qa", name="qa")
            ka = pp.tile([128, KW0], BF16, tag="ka", name="ka")
            va = pp.tile([128, KW0], BF16, tag="va", name="va")
            nc.vector.memset(qa[CH:128, H1_POS:H0_POS], 0.0)
            nc.vector.memset(ka[CH:128, KW1:KW0], 0.0)
            nc.vector.memset(va[CH:128, KW1:KW0], 0.0)

            qs3 = qs[:].rearrange("a (r c) -> a r c", c=WP)
            # central 56x56 of the padded q plane
            nc.sync.dma_start(
                qa[0:CH, :].rearrange("a (x y) -> a x y", y=W),
                qs3[:, 3:3 + H0_ROWS, 3:3 + W])
            nc.sync.dma_start(
                qa[CH:128, 0:H1_POS].rearrange("a (x y) -> a x y", y=W),
                qs3[:, 3 + H0_ROWS:3 + H, 3:3 + W])
            nc.sync.dma_start(ka[0:CH, :], ks[:, 0:KW0])
            nc.sync.dma_start(ka[CH:128, 0:KW1], ks[:, H1_KOFF:NPAD])
            nc.sync.dma_start(va[0:CH, :], vs[:, 0:KW0])
            nc.sync.dma_start(va[CH:128, 0:KW1], vs[:, H1_KOFF:NPAD])

            # odd-element-shifted copies keep every window 4B-aligned so
            # bf16 tensor_tensor stays in 2x mode for odd j shifts
            kao = pp.tile([128, KW0], BF16, tag="kao", name="kao")
            vao = pp.tile([128, KW0], BF16, tag="vao", name="vao")
            nc.scalar.copy(kao[:, 0:KW0 - 1], ka[:, 1:KW0])
            nc.scalar.copy(vao[:, 0:KW0 - 1], va[:, 1:KW0])

            qa32 = pp.tile([128, H0_POS], F32, tag="qa32", name="qa32")
            nc.vector.memset(qa32[CH:128, H1_POS:H0_POS], 0.0)
            qs323 = qs32[:].rearrange("a (r c) -> a r c", c=WP)
            nc.sync.dma_start(
                qa32[0:CH, :].rearrange("a (x y) -> a x y", y=W),
                qs323[:, 3:3 + H0_ROWS, 3:3 + W])
            nc.sync.dma_start(
                qa32[CH:128, 0:H1_POS].rearrange("a (x y) -> a x y", y=W),
                qs323[:, 3 + H0_ROWS:3 + H, 3:3 + W])

            qa3 = qa[:].rearrange("a (x y) -> a x y", y=W)        # [128,32,56]

            def win(t, i, j):
                src_t, jj = (t[0], j) if j % 2 == 0 else (t[1], j - 1)
                t3 = src_t[:].rearrange("a (r c) -> a r c", c=WP)
                return t3[:, i:i + H0_ROWS, jj:jj + W]

            # ---- qk: scores[part, seg, p] (bias added afterwards) ----
            S = pp.tile([128, H0_SEG * NSH], F32, tag="S", name="S")
            S3 = S[:].rearrange("a (s q) -> a s q", q=NSH)
            for p in range(NSH):
                i, j = divmod(p, K)
                prod = wp.tile([128, H0_POS], BF16, tag="prod", name="prod",
                               bufs=2)
                eng = nc.gpsimd if p % 2 == 1 else nc.vector
                eng.tensor_tensor(
                    out=prod[:].rearrange("a (x y) -> a x y", y=W),
                    in0=win((ka, kao), i, j), in1=qa3, op=OP.mult)
                nc.vector.tensor_reduce(
                    out=S3[:, :, p:p + 1],
                    in_=prod[:].rearrange("a (s d) -> a s d", d=SEG),
                    axis=AX.X, op=OP.add)

            # ---- + qsum * bias49 (rank-1), then exp / denominators ----
            qsum = pp.tile([128, H0_SEG], F32, tag="qsum", name="qsum")
            nc.vector.tensor_reduce(
                out=qsum[:],
                in_=qa32[:].rearrange("a (s d) -> a s d", d=SEG),
                axis=AX.X, op=OP.add)
            tb = pp.tile([128, H0_SEG * NSH], F32, tag="tb", name="tb")
            tb3 = tb[:].rearrange("a (s q) -> a s q", q=NSH)
            nc.vector.tensor_tensor(
                out=tb3,
                in0=qsum[:].rearrange("a (s o) -> a s o", o=1).broadcast_to(
                    (128, H0_SEG, NSH)),
                in1=b49_s[:].rearrange("a (o q) -> a o q", o=1).broadcast_to(
                    (128, H0_SEG, NSH)),
                op=OP.mult)
            sb = pp.tile([128, H0_SEG * NSH], F32, tag="sb", name="sb")
            nc.vector.tensor_tensor(out=sb[:], in0=S[:], in1=tb[:], op=OP.add)
            # the rank-1 bias term reaches +-100: must subtract the max
            # before exp or fp32 overflows
            sb3 = sb[:].rearrange("a (s q) -> a s q", q=NSH)
            mx = pp.tile([128, H0_SEG], F32, tag="mx", name="mx")
            nc.vector.tensor_reduce(out=mx[:], in_=sb3, axis=AX.X, op=OP.max)
            nc.vector.tensor_tensor(
                out=sb3, in0=sb3,
                in1=mx[:].rearrange("a (s o) -> a s o", o=1).broadcast_to(
                    (128, H0_SEG, NSH)),
                op=OP.subtract)
            E = pp.tile([128, H0_SEG * NSH], F32, tag="E", name="E")
            nc.scalar.activation(E[:], sb[:], AF.Exp)
            E3 = E[:].rearrange("a (s q) -> a s q", q=NSH)
            den = pp.tile([128, H0_SEG], F32, tag="den", name="den")
            nc.vector.tensor_reduce(out=den[:], in_=E3, axis=AX.X, op=OP.add)
            rcp = pp.tile([128, H0_SEG], F32, tag="rcp", name="rcp")
            nc.vector.reciprocal(rcp[:], den[:])

            # ---- av: acc[part, pos] = sum_p w_p * V_win_p (bf16 chain) ----
            # two independent accumulator chains: DVE owns 37 shifts,
            # GPSIMD owns 12 (p%4==2) end-to-end (mul+add), combined once
            accA = pp.tile([128, H0_POS], BF16, tag="accA", name="accA")
            accB = pp.tile([128, H0_POS], BF16, tag="accB", name="accB")
            accPA = pp.tile([128, H0_POS], BF16, tag="accPA", name="accPA")
            accPB = pp.tile([128, H0_POS], BF16, tag="accPB", name="accPB")
            dve_n = pool_n = 0
            for p in range(NSH):
                i, j = divmod(p, K)
                on_pool = (p % 3 == 2) or (p % 12 == 1)
                eng = nc.gpsimd if on_pool else nc.vector
                wexp = wp.tile([128, H0_POS], BF16, tag="wexp", name="wexp",
                               bufs=3)
                nc.scalar.copy(
                    out=wexp[:].rearrange("a (s d) -> a s d", d=SEG),
                    in_=E3[:, :, p:p + 1].broadcast_to((128, H0_SEG, SEG)))
                wx = wexp[:].rearrange("a (x y) -> a x y", y=W)
                vwin = win((va, vao), i, j)
                if on_pool:
                    first, pair = pool_n == 0, (accPA, accPB)
                    pool_n += 1
                    k_n = pool_n
                else:
                    first, pair = dve_n == 0, (accA, accB)
                    dve_n += 1
                    k_n = dve_n
                if first:
                    eng.tensor_tensor(
                        out=pair[0][:].rearrange("a (x y) -> a x y", y=W),
                        in0=wx, in1=vwin, op=OP.mult)
                else:
                    tag = "avtP" if on_pool else "avt"
                    tmp = wp.tile([128, H0_POS], BF16, tag=tag, name=tag,
                                  bufs=2)
                    eng.tensor_tensor(
                        out=tmp[:].rearrange("a (x y) -> a x y", y=W),
                        in0=wx, in1=vwin, op=OP.mult)
                    src_t, dst = pair if k_n % 2 == 0 else (pair[1], pair[0])
                    eng.tensor_tensor(
                        out=dst[:], in0=src_t[:], in1=tmp[:], op=OP.add)
            accD = accA if dve_n % 2 == 1 else accB
            accP = accPA if pool_n % 2 == 1 else accPB
            acc = accB if dve_n % 2 == 1 else accA
            nc.vector.tensor_tensor(out=acc[:], in0=accD[:], in1=accP[:],
                                    op=OP.add)

            # ---- normalize (fp32 out) and store ----
            fin = pp.tile([128, H0_POS], F32, tag="fin", name="fin")
            rcpb = rcp[:].rearrange("a (s o) -> a s o", o=1).broadcast_to(
                (128, H0_SEG, SEG))
            nc.vector.tensor_tensor(
                out=fin[:].rearrange("a (s d) -> a s d", d=SEG),
                in0=acc[:].rearrange("a (s d) -> a s d", d=SEG),
                in1=rcpb, op=OP.mult)
            nc.sync.dma_start(out_d[:, 0:H0_POS], fin[0:CH, :])
            nc.sync.dma_start(out_d[:, H0_POS:NPOS], fin[CH:128, 0:H1_POS])
    return nc


import json


def _legalize_waits(bir_bytes):
    """Walrus codegen rejects >1 semaphore wait per instruction; hoist the
    extras onto NoOps (same engine, immediately before) so every
    instruction carries at most one wait."""
    bir = json.loads(bir_bytes)
    ctr = [0]

    def fix_block(instructions):
        out = []
        for ins in instructions:
            si = ins.get("sync_info")
            if si:
                w = si.get("on_wait") or []
                if len(w) > 1:
                    for extra in w[:-1]:
                        ctr[0] += 1
                        out.append({
                            "debug": ins.get("debug", 0),
                            "engine": ins["engine"],
                            "ins": [], "outs": [],
                            "name": f"I-lw{ctr[0]}",
                            "opcode": "NoOp",
                            "sync_info": {"on_wait": [extra],
                                          "on_update": []},
                        })
                    si["on_wait"] = [w[-1]]
            out.append(ins)
        instructions[:] = out

    def walk(o):
        if isinstance(o, dict):
            if "instructions" in o:
                fix_block(o["instructions"])
            for v in o.values():
                walk(v)
        elif isinstance(o, list):
            for v in o:
                walk(v)

    walk(bir)
    return json.dumps(bir).encode()


_NC_CACHE = {}


def kernel(x, q_w, q_b, k_w, k_b, v_w, v_b, h_pos, w_pos):
    x = np.asarray(x, np.float32)
    xp = np.pad(x[0], ((0, 0), (3, 3), (3, 3))).reshape(C, NPAD)
    bias49 = (np.asarray(h_pos, np.float32).sum(0)
              + np.asarray(w_pos, np.float32).sum(0)).reshape(NSH)
    b49bc = np.ascontiguousarray(np.tile(bias49[None, :], (128, 1)))

    in_maps = []
    chan_lists = []
    for r in range(N_CORES):
        chans = np.array([64 * h + 8 * r + t for h in range(8)
                          for t in range(8)])
        chan_lists.append(chans)
        wq = np.asarray(q_w, np.float32)[chans, :]
        wk = np.asarray(k_w, np.float32)[chans, :]
        wv = np.asarray(v_w, np.float32)[chans, :]
        wT = np.ascontiguousarray(
            np.concatenate([wq.T, wk.T, wv.T], axis=1))
        bqk = np.concatenate([np.asarray(q_b, np.float32)[chans],
                              np.asarray(k_b, np.float32)[chans]])
        in_maps.append({
            "xp": xp,
            "wT": wT,
            "bqk": np.ascontiguousarray(bqk[:, None]),
            "bv": np.ascontiguousarray(
                np.asarray(v_b, np.float32)[chans][:, None]),
            "b49": b49bc,
        })

    if "nc" not in _NC_CACHE:
        nc = _build_nc()
        legal = _legalize_waits(nc.to_json_bytes())
        nc.to_json_bytes = lambda: legal
        _NC_CACHE["nc"] = nc
    res = run_bass_kernel_spmd(_NC_CACHE["nc"], in_maps,
                               list(range(N_CORES)))
    _NC_CACHE["last_results"] = res

    out = np.empty((C, NPOS), np.float32)
    for r in range(N_CORES):
        out[chan_lists[r], :] = np.asarray(res.results[r]["out"])
    return out.reshape(1, C, H, W)


if __name__ == "__main__":
    _build_nc()
    print("build OK")



# revision 16
# speedup vs baseline: 1.8675x; 1.8675x over previous
"""Trainium2 Bass kernel for nn_Attention_layer_12249246728743.

Reference structure (after untangling the C-order reshape): per channel c
of 512, the 3136 raster positions split into 49 segments of 64
consecutive positions; each segment attends over a 7x7 shifted window of
its OWN channel plane (depthwise local attention):

  scores[c,s,p=(i,j)] = sum_d q[c,64s+d] * k[c, win(64s+d, i, j)]
                        + (sum_d q[c,64s+d]) * bias49[p]
  w = softmax_p(scores);  out[c,64s+d] = sum_p w[c,s,p] * v[c, win(...)]

with q/k/v = 1x1 convs of x (k, v on the zero-padded 62x62 domain).

Sharding: channel-parallel across 8 cores (64 channels each); every
attention segment is core-local: no halo, no collectives.

Layout: "pair-packed" attention - partitions = 64 channels x 2
shift-pair halves, free dim = the full 3136-position raster. The B half
holds k/v planes pre-shifted by +1 (or +56 for row-wrapping pairs), so
ONE tensor op computes two of the 49 window shifts at once (25 ops
instead of 49, no half-image padding waste). QK segment sums use
contiguous-half bf16 add trees (2x DVE mode) + a small TensorReduce
instead of full-width fp32 reduces. AV weights enter the multiply as
stride-0 broadcast APs (free-size-matched zip with the window AP), and
the 25 AV product tiles are accumulated on the otherwise-idle PE via
identity matmuls into PSUM. qsum (rank-1 bias term) is exact host-side
algebra (q_w @ segment-sums-of-x), mirroring the host-collapsed bias49.
Work is split between DVE and GPSIMD by a greedy makespan balancer.
"""

import numpy as np

import concourse.bass as bass
import concourse.mybir as mybir
import concourse.tile as tile
from concourse.bass_utils import run_bass_kernel_spmd

F32 = mybir.dt.float32
BF16 = mybir.dt.bfloat16
AX = mybir.AxisListType
OP = mybir.AluOpType
AF = mybir.ActivationFunctionType

N_CORES = 8
C = 512
H = W = 56
HP = WP = 62          # padded spatial
NPOS = H * W          # 3136
NPAD = HP * WP        # 3844
K = 7
NSH = K * K           # 49 shifts
SEG = 64              # positions per attention segment
NSEG = NPOS // SEG    # 49 segments per channel
CH = 64               # channels per core
NPAIR = 25            # 24 shift pairs + 1 single (p=48)

# shift pairs (pA=2t, pB=2t+1): B half of the k/v tiles is pre-shifted by
# +1 (same-row j->j+1) or +56 (row wrap (i,6)->(i+1,0)). Window slice is
# always pA's (i,j) clamped to j<=6.
def _pair_table():
    pairs = []
    for t in range(24):
        pA = 2 * t
        i, j = divmod(pA, K)
        if j < K - 1:
            pairs.append((i, j, "k1"))     # B = (i, j+1) via +1 tile
        else:
            pairs.append((i, j, "k56"))    # B = (i+1, 0) via +56 tile
    pairs.append((6, 6, "k1"))             # p=48 single; B half masked
    return pairs


PAIRS = _pair_table()

# --- naive op-cost mirror (documented TRN2 formulas) for the balancer ---
def _c_tt(fd, bf16_out=True):
    return (58 + (fd / 2 if bf16_out else fd)) / 0.96


def _c_red(fd_in):
    return (58 + fd_in) / 0.96


class _Sched:
    """Greedy DVE/GPSIMD makespan balancer (Pool runs at 2x cost)."""

    def __init__(self, nc):
        self.nc = nc
        self.v = 0.0
        self.g = 0.0

    def pick(self, cost):
        if max(self.v + cost, self.g) <= max(self.v, self.g + 2 * cost):
            self.v += cost
            return self.nc.vector
        self.g += 2 * cost
        return self.nc.gpsimd


def _build_nc():
    nc = bass.Bass()

    xp = nc.declare_dram_parameter("xp", [C, NPAD], F32, isOutput=False)
    wT = nc.declare_dram_parameter("wT", [C, 3 * CH], F32, isOutput=False)
    bqk = nc.declare_dram_parameter("bqk", [2 * CH, 1], F32, isOutput=False)
    bv = nc.declare_dram_parameter("bv", [CH, 1], F32, isOutput=False)
    bk0 = nc.declare_dram_parameter("bk0", [CH, 1], F32, isOutput=False)
    b49c = nc.declare_dram_parameter("b49c", [128, NPAIR], F32,
                                     isOutput=False)
    qsumd = nc.declare_dram_parameter("qsumd", [128, NSEG], F32,
                                      isOutput=False)
    identp = nc.declare_dram_parameter("identp", [128, 64], F32,
                                       isOutput=False)
    out_d = nc.declare_dram_parameter("out", [CH, NPOS], F32, isOutput=True)

    RPC = 7                      # rows per conv chunk
    NCH = RPC * WP               # 434 conv cols per chunk
    NCHI = RPC * W               # 392 interior positions per chunk
    NCK = 8                      # conv chunks (8*7 = 56 rows)
    AVC = 448                    # AV psum chunk (7 segments)
    NAV = 7                      # AV chunks

    with tile.TileContext(nc) as tc:
        with (
            tc.tile_pool(name="persist", bufs=1) as pp,
            tc.tile_pool(name="work", bufs=2) as wp,
            tc.tile_pool(name="psum", bufs=1, space="PSUM") as psp,
        ):
            sch = _Sched(nc)

            # ---- loads ----
            xt_all = pp.tile([128, 4 * NPAD], F32, tag="xall", name="xall")
            wt_all = pp.tile([128, 4 * 3 * CH], F32, tag="wall", name="wall")
            for s0 in range(0, NPAD, 1024):
                sn = min(1024, NPAD - s0)
                nc.sync.dma_start(
                    xt_all[:].rearrange("p (k n) -> p k n", k=4)[:, :, s0:s0 + sn],
                    xp[:].rearrange("(k p) n -> p k n", p=128)[:, :, s0:s0 + sn])
            nc.sync.dma_start(
                wt_all[:].rearrange("p (k n) -> p k n", k=4),
                wT[:].rearrange("(k p) n -> p k n", p=128))
            xt = [xt_all[:].rearrange("p (k n) -> p k n", k=4)[:, kt, :]
                  for kt in range(4)]
            wt = [wt_all[:].rearrange("p (k n) -> p k n", k=4)[:, kt, :]
                  for kt in range(4)]
            bqk_s = pp.tile([128, 1], F32, tag="bqk", name="bqk")
            bv_s = pp.tile([CH, 1], F32, tag="bv", name="bv")
            bk0_s = pp.tile([CH, 1], F32, tag="bk0", name="bk0")
            nc.sync.dma_start(bk0_s[:], bk0[:])
            b49_s = pp.tile([128, NPAIR], F32, tag="b49", name="b49")
            qsum_s = pp.tile([128, NSEG], F32, tag="qsum", name="qsum")
            idf_s = pp.tile([128, 64], F32, tag="idf", name="idf")
            nc.sync.dma_start(bqk_s[:], bqk[:])
            nc.sync.dma_start(bv_s[:], bv[:])
            nc.sync.dma_start(b49_s[:], b49c[:])
            nc.sync.dma_start(qsum_s[:], qsumd[:])
            nc.sync.dma_start(idf_s[:], identp[:])
            ident = idf_s[:].bitcast(BF16)          # [128, 128] bf16 identity

            # ---- attention-layout staging tiles ----
            qb = pp.tile([128, NPOS], BF16, tag="qb", name="qb")
            k1 = pp.tile([128, NPAD], BF16, tag="k1", name="k1")
            k56 = pp.tile([128, NPAD], BF16, tag="k56", name="k56")
            v1 = pp.tile([128, NPAD], BF16, tag="v1", name="v1")
            v56 = pp.tile([128, NPAD], BF16, tag="v56", name="v56")

            # AV/conv shared PSUM accumulators (7 banks)
            psa = [psp.tile([128, 512], F32, tag=f"psa{j}", name=f"psa{j}")
                   for j in range(NAV)]

            # PE pre-touch (keeps every real Matmult at <=1 sem wait for
            # walrus S3_LW codegen).
            dmy = psp.tile([1, 1], F32, tag="dmy", name="dmy")
            nc.tensor.matmul(dmy[:], lhsT=xt_all[0:1, 0:1],
                             rhs=xt_all[0:1, 0:1], start=True, stop=True)

            # ---- 1x1 convs on interior rows (rows 3..58 of padded) ----
            for ci in range(NCK):
                r0 = 3 + RPC * ci
                pq = psa[(2 * ci) % 6]
                pv = psa[(2 * ci) % 6 + 1]
                for kt in range(4):
                    rhs = xt[kt][:, WP * r0: WP * r0 + NCH]
                    nc.tensor.matmul(pq[:, 0:NCH], lhsT=wt[kt][:, 0:128],
                                     rhs=rhs, start=(kt == 0), stop=(kt == 3))
                    nc.tensor.matmul(pv[0:CH, 0:NCH],
                                     lhsT=wt[kt][:, 128:192],
                                     rhs=rhs, start=(kt == 0), stop=(kt == 3))
                pq3 = pq[:, 0:NCH].rearrange("a (r c) -> a r c", c=WP)
                pv3 = pv[0:CH, 0:NCH].rearrange("a (r c) -> a r c", c=WP)
                qbv = qb[:, NCHI * ci: NCHI * (ci + 1)].rearrange(
                    "a (r c) -> a r c", c=W)
                nc.scalar.activation(qbv[0:CH], pq3[0:CH, :, 3:59],
                                     AF.Identity, bias=bqk_s[0:CH, :])
                nc.scalar.activation(qbv[CH:128], pq3[0:CH, :, 3:59],
                                     AF.Identity, bias=bqk_s[0:CH, :])
                k13 = k1[0:CH, :].rearrange("a (r c) -> a r c", c=WP)
                v13 = v1[0:CH, :].rearrange("a (r c) -> a r c", c=WP)
                nc.scalar.activation(k13[:, r0:r0 + RPC, 3:59],
                                     pq3[CH:128, :, 3:59], AF.Identity,
                                     bias=bqk_s[CH:128, :])
                nc.scalar.activation(v13[:, r0:r0 + RPC, 3:59],
                                     pv3[:, :, 3:59], AF.Identity,
                                     bias=bv_s[:])

            # ---- k/v padded borders hold conv(0)+bias = bias ----
            zt = pp.tile([CH, 1], BF16, tag="zt", name="zt")
            nc.vector.memset(zt[:], 0.0)
            sch.v += (58 + 1) / 0.96
            for plane, bias in ((k1, bk0_s[:]), (v1, bv_s[:])):
                p3 = plane[0:CH, :].rearrange("a (r c) -> a r c", c=WP)
                for view in (
                    plane[0:CH, 0: 3 * WP],                # rows 0-2
                    plane[0:CH, 59 * WP: NPAD],            # rows 59-61
                    p3[:, 3:59, 0:3],                      # left cols
                    p3[:, 3:59, 59:62],                    # right cols
                ):
                    sh = view.shape
                    if len(sh) == 2:
                        zin = zt[:, 0:1].broadcast_to((CH, sh[1]))
                    else:
                        zin = zt[:, 0:1].rearrange(
                            "a (x y) -> a x y", y=1).broadcast_to(
                            (CH, sh[1], sh[2]))
                    nc.scalar.activation(view, zin, AF.Identity, bias=bias)

            # ---- stage shifted B halves (+1 / +56) and k56/v56 A halves --
            for src, d1, d56 in ((k1, k1, k56), (v1, v1, v56)):
                nc.scalar.copy(d1[CH:128, 0:NPAD - 1], src[0:CH, 1:NPAD])
                nc.scalar.copy(d56[0:CH, :], src[0:CH, :])
                nc.scalar.copy(d56[CH:128, 0:NPAD - 56], src[0:CH, 56:NPAD])
                nc.vector.memset(d1[CH:128, NPAD - 1: NPAD], 0.0)
                nc.vector.memset(d56[CH:128, NPAD - 56: NPAD], 0.0)
                sch.v += (58 + 1) / 0.96 + (58 + 56) / 0.96

            # ---- QK: 25 pair products + bf16 add trees -> scores ----
            S_all = pp.tile([128, NPAIR * NSEG], F32, tag="Sall", name="Sall")
            qb3 = qb[:].rearrange("a (x y) -> a x y", y=W)

            def winv(t, i, j):
                t3 = t[:].rearrange("a (r c) -> a r c", c=WP)
                return t3[:, i:i + H, j:j + W]

            CHAIN = (_c_tt(NPOS) + _c_tt(NPOS // 2) + _c_tt(NPOS // 4)
                     + _c_tt(NPOS // 8) + _c_tt(NPOS // 16)
                     + _c_tt(NPOS // 32) + _c_tt(NPOS // 64, False))
            for q in range(NPAIR):
                i, j, tk = PAIRS[q]
                ktile = k1 if tk == "k1" else k56
                eng = sch.pick(CHAIN)
                prod = wp.tile([128, NPOS], BF16, tag="prod", name="prod",
                               bufs=3)
                eng.tensor_tensor(
                    out=prod[:].rearrange("a (x y) -> a x y", y=W),
                    in0=winv(ktile, i, j), in1=qb3, op=OP.mult)
                cur = prod
                w = SEG
                while w > 2:
                    w //= 2
                    t = wp.tile([128, NSEG * w], BF16, tag=f"t{w}",
                                name=f"t{w}", bufs=2)
                    c3 = cur[:].rearrange("a (s d) -> a s d", d=2 * w)
                    eng.tensor_tensor(
                        out=t[:].rearrange("a (s d) -> a s d", d=w),
                        in0=c3[:, :, 0:w], in1=c3[:, :, w:2 * w], op=OP.add)
                    cur = t
                sp = wp.tile([128, NSEG], F32, tag="sp", name="sp", bufs=2)
                c3 = cur[:].rearrange("a (s d) -> a s d", d=2)
                eng.tensor_tensor(
                    out=sp[:].rearrange("a (s o) -> a s o", o=1),
                    in0=c3[:, :, 0:1], in1=c3[:, :, 1:2], op=OP.add)
                # S = qsum*b49[p] + segsum  (rank-1 positional bias).
                # scalar_tensor_tensor fails Pool codegen: pin to DVE.
                sl = slice(NSEG * q, NSEG * (q + 1))
                sch.v += (58 + NSEG) / 0.96
                if q < NPAIR - 1:
                    nc.vector.scalar_tensor_tensor(
                        out=S_all[:, sl], in0=qsum_s[:],
                        scalar=b49_s[:, q:q + 1], in1=sp[:],
                        op0=OP.mult, op1=OP.add)
                else:
                    nc.vector.scalar_tensor_tensor(
                        out=S_all[0:CH, sl], in0=qsum_s[0:CH, :],
                        scalar=b49_s[0:CH, q:q + 1], in1=sp[0:CH, :],
                        op0=OP.mult, op1=OP.add)
            # mask the unused B half of the single shift p=48
            nc.vector.memset(S_all[CH:128, NSEG * 24: NSEG * 25], -1e30)
            sch.v += (58 + NSEG) / 0.96

            # ---- softmax over the 49 shifts (A/B halves + 25 columns) ----
            # (TensorTensor needs equal SBUF base partitions: remap the B
            # half to base 0 via single-input scalar copies first.)
            NQ = NPAIR * NSEG
            sbh = pp.tile([CH, NQ], F32, tag="sbh", name="sbh")
            nc.scalar.copy(sbh[:], S_all[CH:128, :])
            mxh = pp.tile([CH, NQ], F32, tag="mxh", name="mxh")
            sch.v += _c_tt(NQ, False)     # Pool rejects max: pin to DVE
            nc.vector.tensor_tensor(out=mxh[:], in0=S_all[0:CH, :],
                                    in1=sbh[:], op=OP.max)
            mxd = pp.tile([128, NSEG], F32, tag="mxd", name="mxd")
            sch.v += _c_red(NQ)
            nc.vector.tensor_reduce(
                out=mxd[0:CH, :],
                in_=mxh[:].rearrange("a (q s) -> a s q", s=NSEG),
                axis=AX.X, op=OP.max)
            nc.scalar.copy(mxd[CH:128, :], mxd[0:CH, :])
            sb = pp.tile([128, NQ], F32, tag="sb", name="sb")
            eng = sch.pick(_c_tt(NQ, False))
            eng.tensor_tensor(
                out=sb[:].rearrange("a (q s) -> a q s", s=NSEG),
                in0=S_all[:].rearrange("a (q s) -> a q s", s=NSEG),
                in1=mxd[:].rearrange("a (o s) -> a o s", o=1).broadcast_to(
                    (128, NPAIR, NSEG)),
                op=OP.subtract)
            E = pp.tile([128, NQ], BF16, tag="E", name="E")
            nc.scalar.activation(E[:], sb[:], AF.Exp)
            ebh = pp.tile([CH, NQ], BF16, tag="ebh", name="ebh")
            nc.scalar.copy(ebh[:], E[CH:128, :])
            dh = pp.tile([CH, NQ], F32, tag="dh", name="dh")
            eng = sch.pick(_c_tt(NQ, False))
            eng.tensor_tensor(out=dh[:], in0=E[0:CH, :], in1=ebh[:],
                              op=OP.add)
            den = pp.tile([CH, NSEG], F32, tag="den", name="den")
            sch.v += _c_red(NQ)
            nc.vector.tensor_reduce(
                out=den[:], in_=dh[:].rearrange("a (q s) -> a s q", s=NSEG),
                axis=AX.X, op=OP.add)
            rcp = pp.tile([CH, NSEG], F32, tag="rcp", name="rcp")
            nc.vector.reciprocal(rcp[:], den[:])
            sch.v += (58 + NSEG) / 0.96

            # ---- AV: weight-broadcast multiplies + PE accumulation ----
            for q in range(NPAIR):
                i, j, tk = PAIRS[q]
                vtile = v1 if tk == "k1" else v56
                eng = sch.pick(_c_tt(NPOS))
                vp = wp.tile([128, NPOS], BF16, tag="vp", name="vp", bufs=3)
                eng.tensor_tensor(
                    out=vp[:].rearrange("a (s d) -> a s d", d=SEG),
                    in0=winv(vtile, i, j),
                    in1=E[:, NSEG * q: NSEG * (q + 1)].rearrange(
                        "a (s o) -> a s o", o=1).broadcast_to(
                        (128, NSEG, SEG)),
                    op=OP.mult)
                for kch in range(NAV):
                    nc.tensor.matmul(
                        psa[kch][:, 0:AVC], lhsT=ident,
                        rhs=vp[:, AVC * kch: AVC * (kch + 1)],
                        start=(q == 0), stop=(q == NPAIR - 1))

            # ---- normalize, combine halves, store ----
            # (per-half PSUM reads land both halves at base partition 0;
            # PSUM input exempts the equal-SBUF-base TensorTensor rule)
            finA = pp.tile([CH, NPOS], BF16, tag="finA", name="finA")
            finB = pp.tile([CH, NPOS], BF16, tag="finB", name="finB")
            for kch in range(NAV):
                csl = slice(AVC * kch, AVC * (kch + 1))
                rbc = rcp[:, 7 * kch: 7 * (kch + 1)].rearrange(
                    "a (s o) -> a s o", o=1).broadcast_to((CH, 7, SEG))
                for half, ftile in ((0, finA), (1, finB)):
                    # gpsimd cannot read PSUM: pin to DVE
                    sch.v += _c_tt(AVC)
                    nc.vector.tensor_tensor(
                        out=ftile[:, csl].rearrange("a (s d) -> a s d",
                                                    d=SEG),
                        in0=psa[kch][CH * half: CH * (half + 1),
                                     0:AVC].rearrange(
                            "a (s d) -> a s d", d=SEG),
                        in1=rbc, op=OP.mult)
            fout = pp.tile([CH, NPOS], F32, tag="fout", name="fout")
            eng = sch.pick(_c_tt(NPOS, False))
            eng.tensor_tensor(out=fout[:], in0=finA[:], in1=finB[:],
                              op=OP.add)
            nc.sync.dma_start(out_d[:], fout[:])
    return nc


import json


def _legalize_waits(bir_bytes):
    """Walrus codegen rejects >1 semaphore wait per instruction; hoist the
    extras onto NoOps (same engine, immediately before) so every
    instruction carries at most one wait."""
    bir = json.loads(bir_bytes)
    ctr = [0]

    def fix_block(instructions):
        out = []
        for ins in instructions:
            si = ins.get("sync_info")
            if si:
                w = si.get("on_wait") or []
                if len(w) > 1:
                    for extra in w[:-1]:
                        ctr[0] += 1
                        out.append({
                            "debug": ins.get("debug", 0),
                            "engine": ins["engine"],
                            "ins": [], "outs": [],
                            "name": f"I-lw{ctr[0]}",
                            "opcode": "NoOp",
                            "sync_info": {"on_wait": [extra],
                                          "on_update": []},
                        })
                    si["on_wait"] = [w[-1]]
            out.append(ins)
        instructions[:] = out

    def walk(o):
        if isinstance(o, dict):
            if "instructions" in o:
                fix_block(o["instructions"])
            for v in o.values():
                walk(v)
        elif isinstance(o, list):
            for v in o:
                walk(v)

    walk(bir)
    return json.dumps(bir).encode()


_NC_CACHE = {}


def kernel(x, q_w, q_b, k_w, k_b, v_w, v_b, h_pos, w_pos):
    x = np.asarray(x, np.float64)
    xp32 = np.pad(x[0], ((0, 0), (3, 3), (3, 3))).reshape(C, NPAD).astype(
        np.float32)
    bias49 = (np.asarray(h_pos, np.float64).sum(0)
              + np.asarray(w_pos, np.float64).sum(0)).reshape(NSH)
    # per-pair positional-bias column: rows 0-63 = bias49[2q], 64-127 =
    # bias49[2q+1] (0 for the unused B half of the last single)
    b49cols = np.zeros((128, NPAIR), np.float64)
    for q in range(NPAIR):
        b49cols[0:CH, q] = bias49[2 * q]
        if q < NPAIR - 1:
            b49cols[CH:128, q] = bias49[2 * q + 1]
    b49cols = np.ascontiguousarray(b49cols.astype(np.float32))

    # exact host qsum: sum_d q[c,seg] = q_w[c,:] @ (seg-sums of x) + 64*q_b
    xs = x[0].reshape(C, NSEG, SEG).sum(-1)              # [512, 49] f64
    qsum_all = (np.asarray(q_w, np.float64) @ xs
                + 64.0 * np.asarray(q_b, np.float64)[:, None])  # [512, 49]

    eye_u16 = (np.eye(128, dtype=np.uint16) * 0x3F80)    # bf16 1.0
    identp = np.ascontiguousarray(eye_u16.view(np.float32))

    in_maps = []
    chan_lists = []
    for r in range(N_CORES):
        chans = np.array([64 * h + 8 * r + t for h in range(8)
                          for t in range(8)])
        chan_lists.append(chans)
        wq = np.asarray(q_w, np.float32)[chans, :]
        wk = np.asarray(k_w, np.float32)[chans, :]
        wv = np.asarray(v_w, np.float32)[chans, :]
        wTl = np.ascontiguousarray(
            np.concatenate([wq.T, wk.T, wv.T], axis=1))
        bqk = np.concatenate([np.asarray(q_b, np.float32)[chans],
                              np.asarray(k_b, np.float32)[chans]])
        qsd = np.ascontiguousarray(
            np.tile(qsum_all[chans].astype(np.float32), (2, 1)))
        in_maps.append({
            "xp": xp32,
            "wT": wTl,
            "bqk": np.ascontiguousarray(bqk[:, None]),
            "bv": np.ascontiguousarray(
                np.asarray(v_b, np.float32)[chans][:, None]),
            "bk0": np.ascontiguousarray(
                np.asarray(k_b, np.float32)[chans][:, None]),
            "b49c": b49cols,
            "qsumd": qsd,
            "identp": identp,
        })

    if "nc" not in _NC_CACHE:
        nc = _build_nc()
        legal = _legalize_waits(nc.to_json_bytes())
        nc.to_json_bytes = lambda: legal
        _NC_CACHE["nc"] = nc
    res = run_bass_kernel_spmd(_NC_CACHE["nc"], in_maps,
                               list(range(N_CORES)))
    _NC_CACHE["last_results"] = res

    out = np.empty((C, NPOS), np.float32)
    for r in range(N_CORES):
        out[chan_lists[r], :] = np.asarray(res.results[r]["out"])
    return out.reshape(1, C, H, W)


if __name__ == "__main__":
    _build_nc()
    print("build OK")
